# revision 1
# baseline (speedup 1.0000x reference)
import numpy as np
import ml_dtypes

import concourse.bacc as bacc
import concourse.tile as tile
from concourse import mybir

# Problem: NIMSCrossEntropyLoss
#   preds (4, 4, 4, 512, 512) f32, targets (4, 4, 512, 512) int32
#   Only the S=-1 slice contributes:
#   loss = [sum_pixels logsumexp_c(p) - sum_pixels p[target]] / N_BATCH
# Shard the 4*512*512 = 1048576 pixels over 8 cores:
#   131072 pixels/core as [128 partitions, 1024 free] channel planes (bf16).
# v3: per-plane DRAM tensors + 3 parallel DMA queues (ACT/SP/SWDGE) +
#     per-plane exp and a DVE order that feeds ln as early as possible.

N_CORES = 8
P = 128           # partitions
C = 4             # classes
N_BATCH = 4       # reference divides by this
F = 1024          # pixels per partition per core

BF16 = mybir.dt.bfloat16
F32 = mybir.dt.float32

_PATCHED = False


def _patch_act_tables():
    """Force exp+ln into the combined ACT table so only one table load is
    emitted (greedy per-function set choice otherwise alternates sets)."""
    global _PATCHED
    if _PATCHED:
        return
    import concourse.hw_specs as hw_specs
    real = hw_specs.get_activation_tables
    Exp = mybir.ActivationFunctionType.Exp
    Ln = mybir.ActivationFunctionType.Ln

    def patched(arch):
        out = {}
        for name, fns in dict(real(arch)).items():
            if name != "natural_log_exp_and_others":
                fns = fns - {Exp, Ln}
            out[name] = fns
        return out

    bacc.get_activation_tables = patched
    _PATCHED = True


def build_nc(f=F, finalize=True):
    """One core's shard: p0..p3 channel planes [P, f] bf16, tgt [P, f] bf16;
    out [P, 5] f32 = per-partition sums (p_t for c=0..3, lse)."""
    _patch_act_tables()
    nc = bacc.Bacc("TRN2", target_bir_lowering=False, debug=False)
    planes = [nc.dram_tensor(f"p{c}", (P, f), BF16, kind="ExternalInput").ap()
              for c in range(C)]
    tgt = nc.dram_tensor("tgt", (P, f), BF16, kind="ExternalInput").ap()
    out = nc.dram_tensor("out", (P, 5), F32, kind="ExternalOutput").ap()

    Exp = mybir.ActivationFunctionType.Exp
    Ln = mybir.ActivationFunctionType.Ln

    with tile.TileContext(nc) as tc:
        with tc.tile_pool(name="w", bufs=1) as w:
            pt = [w.tile([P, f], BF16, name=f"pt{c}") for c in range(C)]
            tt = w.tile([P, f], BF16)

            # Sync + GpSimd DMA queues only: scalar.dma_start forces a
            # spurious extra ACT table load whose DRAM traffic starves the
            # input DMAs. Interleaved completion -> p0, tgt, p1, p2, p3.
            # (Splitting tgt/p0 into half-transfers was tried and is slower:
            # extra issue overhead pushes the ACT table load late, and a
            # concurrent gpsimd add causes SBUF contention that slows DVE.)
            nc.sync.dma_start(out=pt[0], in_=planes[0])
            nc.gpsimd.dma_start(out=tt, in_=tgt)
            nc.sync.dma_start(out=pt[1], in_=planes[1])
            nc.gpsimd.dma_start(out=pt[2], in_=planes[2])
            nc.sync.dma_start(out=pt[3], in_=planes[3])

            res = w.tile([P, 5], F32)
            e = [w.tile([P, f], BF16, name=f"e{c}") for c in range(C)]
            for c in range(C):
                nc.scalar.activation(out=e[c], in_=pt[c], func=Exp)

            scr = w.tile([P, 4 * f], BF16)

            def stt(c):
                nc.vector.scalar_tensor_tensor(
                    out=scr[:, c * f:(c + 1) * f], in0=tt, scalar=float(c),
                    in1=pt[c],
                    op0=mybir.AluOpType.is_equal, op1=mybir.AluOpType.mult,
                    accum_out=res[:, c:c + 1],
                )

            s01 = w.tile([P, f], BF16)
            s012 = w.tile([P, f], BF16)
            s = w.tile([P, f], BF16)

            # The scheduler batches all 4 stts first on DVE regardless of
            # emission order (priority hints don't change it), then runs the
            # three adds and ln.
            stt(0)
            stt(1)
            nc.vector.tensor_tensor(out=s01, in0=e[0], in1=e[1],
                                    op=mybir.AluOpType.add)
            stt(2)
            nc.vector.tensor_tensor(out=s012, in0=s01, in1=e[2],
                                    op=mybir.AluOpType.add)
            nc.vector.tensor_tensor(out=s, in0=s012, in1=e[3],
                                    op=mybir.AluOpType.add)
            stt(3)

            lnout = w.tile([P, f], BF16)
            nc.scalar.activation(out=lnout, in_=s, func=Ln,
                                 accum_out=res[:, 4:5])

            nc.sync.dma_start(out=out, in_=res)
    if finalize:
        nc.finalize()
    return nc


_NC_CACHE = {}


def _get_nc(f=F):
    if f not in _NC_CACHE:
        _NC_CACHE[f] = build_nc(f)
    return _NC_CACHE[f]


def prep_inputs(preds, targets):
    """Host-side shard prep: S=-1 slice, per-channel planes, 8-way split."""
    p = np.asarray(preds)[:, -1]       # (N=4, C=4, 512, 512) f32
    t = np.asarray(targets)[:, -1]     # (4, 512, 512) int
    arr = np.transpose(p, (1, 0, 2, 3)).reshape(C, N_CORES, P, -1)
    arr = arr.astype(ml_dtypes.bfloat16)
    tf = t.reshape(N_CORES, P, -1).astype(ml_dtypes.bfloat16)
    maps = []
    for k in range(N_CORES):
        m = {f"p{c}": np.ascontiguousarray(arr[c, k]) for c in range(C)}
        m["tgt"] = tf[k]
        maps.append(m)
    return maps


def reduce_outputs(results):
    total = 0.0
    for d in results:
        o = d["out"].astype(np.float64)
        total += float(o[:, 4].sum() - o[:, 0:4].sum())
    return np.float32(total / N_BATCH)


def kernel(preds, targets, _trace=False, _trace_kwargs=None):
    from concourse.bass_utils import run_bass_kernel_spmd

    in_maps = prep_inputs(preds, targets)
    f = in_maps[0]["tgt"].shape[1]
    nc = _get_nc(f=f)
    r = run_bass_kernel_spmd(
        nc, in_maps, core_ids=list(range(N_CORES)),
        trace=_trace, **(_trace_kwargs or {}),
    )
    kernel.last_run = r
    return reduce_outputs(r.results)


kernel.last_run = None



# revision 4
# speedup vs baseline: 1.3469x; 1.3469x over previous
import numpy as np
import ml_dtypes

import concourse.bacc as bacc
import concourse.tile as tile
from concourse import mybir

# NIMSCrossEntropyLoss: loss = [sum_px lse_c(p) - sum_px p[tgt]]/4, S=-1.
# v10: "layout B" — channels on partitions. Per core 131072 px = 32 rows
# x 4096 cols; partition p = c*32 + r holds channel c of pixel row r.
#   inpP  fp8 [128, 4096]  preds  (two 2048-col chunks, one per queue)
#   inpT  fp8 [128, 4096]  target replicated to all 4 channel groups
#   inpW  bf16 [128, 32]   W[p, m] = (p%32 == m)  — ones-block weights
#   inpC  f32 [128, 1]     cvec[p] = p//32        — per-partition channel id
# ACT: exp per chunk; PE: channel-sum via 4 tile-positioned matmuls per
# chunk into stacked PSUM [128, 512]; ACT: ln(psum)+accum per chunk.
# DVE: one mask-dot stt per chunk: (Trep == cvec) * P, accum.

N_CORES = 8
P = 128
C = 4
N_BATCH = 4
FD = 4096         # free dim per core (32 rows x 4096 = 131072 px)
HD = FD // 2      # chunk size

FP8 = mybir.dt.float8e4
BF16 = mybir.dt.bfloat16
F32 = mybir.dt.float32

_PATCHED = False


def _patch_act_tables():
    global _PATCHED
    if _PATCHED:
        return
    import concourse.hw_specs as hw_specs
    real = hw_specs.get_activation_tables
    Exp = mybir.ActivationFunctionType.Exp
    Ln = mybir.ActivationFunctionType.Ln

    def patched(arch):
        out = {}
        for name, fns in dict(real(arch)).items():
            if name != "natural_log_exp_and_others":
                fns = fns - {Exp, Ln}
            out[name] = fns
        return out

    bacc.get_activation_tables = patched
    _PATCHED = True


def build_nc(finalize=True):
    """out [P, 4] f32: col0/1 = mask-dot accums (chunk 1/2),
    col2/3 = ln accums (chunk 1/2)."""
    _patch_act_tables()
    nc = bacc.Bacc("TRN2", target_bir_lowering=False, debug=False)
    inpP1 = nc.dram_tensor("inpP1", (P, HD), FP8, kind="ExternalInput").ap()
    inpP2 = nc.dram_tensor("inpP2", (P, HD), FP8, kind="ExternalInput").ap()
    inpT1 = nc.dram_tensor("inpT1", (P, HD), FP8, kind="ExternalInput").ap()
    inpT2 = nc.dram_tensor("inpT2", (P, HD), FP8, kind="ExternalInput").ap()
    inpW = nc.dram_tensor("inpW", (P, 32), BF16, kind="ExternalInput").ap()
    inpC = nc.dram_tensor("inpC", (P, 1), F32, kind="ExternalInput").ap()
    out = nc.dram_tensor("out", (P, 4), F32, kind="ExternalOutput").ap()

    Exp = mybir.ActivationFunctionType.Exp
    Ln = mybir.ActivationFunctionType.Ln

    with tile.TileContext(nc) as tc:
        with tc.tile_pool(name="w", bufs=1) as w, \
             tc.tile_pool(name="ps", bufs=1, space="PSUM") as ps:
            tP1 = w.tile([P, HD], FP8, name="tP1")
            tP2 = w.tile([P, HD], FP8, name="tP2")
            tT1 = w.tile([P, HD], FP8, name="tT1")
            tT2 = w.tile([P, HD], FP8, name="tT2")
            tW = w.tile([P, 32], BF16, name="tW")
            tC = w.tile([P, 1], F32, name="tC")

            # sync/HWDGE: preds chunk1 then out at the end.
            # gpsimd/SWDGE: W+cvec (tiny), Trep1, P2, Trep2.
            nc.sync.dma_start(out=tP1, in_=inpP1)
            nc.gpsimd.dma_start(out=tW, in_=inpW)
            nc.gpsimd.dma_start(out=tC, in_=inpC)
            nc.gpsimd.dma_start(out=tT1, in_=inpT1)
            nc.gpsimd.dma_start(out=tP2, in_=inpP2)
            nc.gpsimd.dma_start(out=tT2, in_=inpT2)

            res = w.tile([P, 4], F32, name="res")
            e1 = w.tile([P, HD], BF16, name="e1")
            e2 = w.tile([P, HD], BF16, name="e2")
            psum = ps.tile([P, FD // 4], F32, name="psum")  # [128, 1024]

            # PE warmup to raise p-state before the real matmuls
            warm = ps.tile([P, 32], F32, name="warm")
            nc.tensor.matmul(out=warm[0:32, :], lhsT=tW, rhs=tW,
                             start=True, stop=True, tile_position=(0, 0))

            nc.scalar.activation(out=e1, in_=tP1, func=Exp)
            nc.scalar.activation(out=e2, in_=tP2, func=Exp)

            Q = HD // 4  # 512 cols per sub-matmul
            for q in range(4):
                nc.tensor.matmul(out=psum[q * 32:(q + 1) * 32, 0:Q],
                                 lhsT=tW, rhs=e1[:, q * Q:(q + 1) * Q],
                                 start=True, stop=True, tile_position=(0, q * 32))
            for q in range(4):
                nc.tensor.matmul(out=psum[q * 32:(q + 1) * 32, Q:2 * Q],
                                 lhsT=tW, rhs=e2[:, q * Q:(q + 1) * Q],
                                 start=True, stop=True, tile_position=(0, q * 32))

            # mask-dots: (Trep == cvec) * P, one stt per chunk
            scr = w.tile([P, HD], BF16, name="scr")
            nc.vector.scalar_tensor_tensor(
                out=scr, in0=tT1, scalar=tC[:, 0:1], in1=tP1,
                op0=mybir.AluOpType.is_equal, op1=mybir.AluOpType.mult,
                accum_out=res[:, 0:1],
            )
            nc.vector.scalar_tensor_tensor(
                out=scr, in0=tT2, scalar=tC[:, 0:1], in1=tP2,
                op0=mybir.AluOpType.is_equal, op1=mybir.AluOpType.mult,
                accum_out=res[:, 1:2],
            )

            lnout = w.tile([P, Q], BF16, name="lnout")
            nc.scalar.activation(out=lnout, in_=psum[:, 0:Q], func=Ln,
                                 accum_out=res[:, 2:3])
            nc.scalar.activation(out=lnout, in_=psum[:, Q:2 * Q], func=Ln,
                                 accum_out=res[:, 3:4])

            nc.sync.dma_start(out=out, in_=res)
    if finalize:
        nc.finalize()
    return nc


_NC_CACHE = {}


def _get_nc():
    if "nc" not in _NC_CACHE:
        _NC_CACHE["nc"] = build_nc()
    return _NC_CACHE["nc"]


def prep_inputs(preds, targets):
    """Layout B: core k gets flat pixels [k*131072, (k+1)*131072) as
    [32 rows, 4096 cols]; partition c*32+r = channel c, row r."""
    p = np.asarray(preds)[:, -1]       # (4, 4, 512, 512) f32
    t = np.asarray(targets)[:, -1]     # (4, 512, 512) int
    # (C, N_CORES, 32, 4096)
    arr = np.transpose(p, (1, 0, 2, 3)).reshape(C, N_CORES, 32, FD)
    arr = arr.astype(ml_dtypes.float8_e4m3)
    tf = t.reshape(N_CORES, 32, FD).astype(ml_dtypes.float8_e4m3)
    W = np.zeros((P, 32), dtype=ml_dtypes.bfloat16)
    for pp in range(P):
        W[pp, pp % 32] = 1.0
    cvec = (np.arange(P, dtype=np.float32) // 32)[:, None]
    maps = []
    for k in range(N_CORES):
        pb = arr[:, k].reshape(P, FD)          # partition c*32+r
        trep = np.tile(tf[k], (4, 1))          # [128, 4096]
        maps.append({
            "inpP1": np.ascontiguousarray(pb[:, 0:HD]),
            "inpP2": np.ascontiguousarray(pb[:, HD:FD]),
            "inpT1": np.ascontiguousarray(trep[:, 0:HD]),
            "inpT2": np.ascontiguousarray(trep[:, HD:FD]),
            "inpW": W,
            "inpC": cvec,
        })
    return maps


def reduce_outputs(results):
    total = 0.0
    for d in results:
        o = d["out"].astype(np.float64)
        total += float(o[:, 2:4].sum() - o[:, 0:2].sum())
    return np.float32(total / N_BATCH)


def kernel(preds, targets, _trace=False, _trace_kwargs=None):
    from concourse.bass_utils import run_bass_kernel_spmd

    in_maps = prep_inputs(preds, targets)
    nc = _get_nc()
    r = run_bass_kernel_spmd(
        nc, in_maps, core_ids=list(range(N_CORES)),
        trace=_trace, **(_trace_kwargs or {}),
    )
    kernel.last_run = r
    return reduce_outputs(r.results)


kernel.last_run = None


# revision 5
# speedup vs baseline: 1.3479x; 1.0007x over previous
import numpy as np
import ml_dtypes

import concourse.bacc as bacc
import concourse.tile as tile
from concourse import mybir

# NIMSCrossEntropyLoss: loss = [sum_px lse_c(p) - sum_px p[tgt]]/4, S=-1.
# v11: "layout B" — channels on partitions. Per core 131072 px = 32 rows
# x 4096 cols; partition p = c*32 + r holds channel c of pixel row r.
#   inpP  fp8 [128, 4096]  preds  (two 2048-col chunks, one per queue)
#   inpT  fp8 [128, 4096]  target replicated to all 4 channel groups
#   inpW  bf16 [128, 32]   W[p, m] = (p%32 == m)  — ones-block weights
#   inpC  f32 [128, 1]     cvec[p] = p//32        — per-partition channel id
# ACT: exp per chunk; PE: channel-sum via 4 tile-positioned matmuls per
# chunk into stacked PSUM [128, 512]; ACT: ln(psum)+accum per chunk.
# DVE: one mask-dot stt per chunk: (Trep == cvec) * P, accum.

N_CORES = 8
P = 128
C = 4
N_BATCH = 4
FD = 4096         # free dim per core (32 rows x 4096 = 131072 px)
HD = FD // 2      # chunk size

FP8 = mybir.dt.float8e4
BF16 = mybir.dt.bfloat16
F32 = mybir.dt.float32

_PATCHED = False


def _patch_act_tables():
    global _PATCHED
    if _PATCHED:
        return
    import concourse.hw_specs as hw_specs
    real = hw_specs.get_activation_tables
    Exp = mybir.ActivationFunctionType.Exp
    Ln = mybir.ActivationFunctionType.Ln

    def patched(arch):
        out = {}
        for name, fns in dict(real(arch)).items():
            if name != "natural_log_exp_and_others":
                fns = fns - {Exp, Ln}
            out[name] = fns
        return out

    bacc.get_activation_tables = patched
    _PATCHED = True


def build_nc(finalize=True):
    """out [P, 4] f32: col0/1 = mask-dot accums (chunk 1/2),
    col2/3 = ln accums (chunk 1/2)."""
    _patch_act_tables()
    nc = bacc.Bacc("TRN2", target_bir_lowering=False, debug=False)
    inpP1 = nc.dram_tensor("inpP1", (P, HD), FP8, kind="ExternalInput").ap()
    inpP2 = nc.dram_tensor("inpP2", (P, HD), FP8, kind="ExternalInput").ap()
    inpT1 = nc.dram_tensor("inpT1", (P, HD), FP8, kind="ExternalInput").ap()
    inpT2 = nc.dram_tensor("inpT2", (P, HD), FP8, kind="ExternalInput").ap()
    inpW = nc.dram_tensor("inpW", (P, 32), BF16, kind="ExternalInput").ap()
    inpC = nc.dram_tensor("inpC", (P, 1), F32, kind="ExternalInput").ap()
    out = nc.dram_tensor("out", (P, 4), F32, kind="ExternalOutput").ap()

    Exp = mybir.ActivationFunctionType.Exp
    Ln = mybir.ActivationFunctionType.Ln

    with tile.TileContext(nc) as tc:
        with tc.tile_pool(name="w", bufs=1) as w, \
             tc.tile_pool(name="ps", bufs=1, space="PSUM") as ps:
            tP1 = w.tile([P, HD], FP8, name="tP1")
            tP2 = w.tile([P, HD], FP8, name="tP2")
            tT1 = w.tile([P, HD], FP8, name="tT1")
            tT2 = w.tile([P, HD], FP8, name="tT2")
            tW = w.tile([P, 32], BF16, name="tW")
            tC = w.tile([P, 1], F32, name="tC")

            # sync/HWDGE: cvec (tiny), preds chunks, out at the end.
            # gpsimd/SWDGE: W (tiny), target-replica chunks.
            nc.sync.dma_start(out=tC, in_=inpC)
            nc.sync.dma_start(out=tP1, in_=inpP1)
            nc.sync.dma_start(out=tP2, in_=inpP2)
            nc.gpsimd.dma_start(out=tW, in_=inpW)
            nc.gpsimd.dma_start(out=tT1, in_=inpT1)
            nc.gpsimd.dma_start(out=tT2, in_=inpT2)

            res = w.tile([P, 4], F32, name="res")
            e1 = w.tile([P, HD], BF16, name="e1")
            e2 = w.tile([P, HD], BF16, name="e2")
            psumA = ps.tile([P, FD // 8], F32, name="psumA")  # [128, 512]
            psumB = ps.tile([P, FD // 8], F32, name="psumB")

            # PE heater: junk matmuls keep the tensor engine busy so the
            # real channel-sum matmuls run at a ramped p-state.
            warm = ps.tile([P, 32], F32, name="warm")
            for _ in range(24):
                nc.tensor.matmul(out=warm[0:32, :], lhsT=tW, rhs=tW,
                                 start=True, stop=True, tile_position=(0, 0))

            nc.scalar.activation(out=e1, in_=tP1, func=Exp)
            nc.scalar.activation(out=e2, in_=tP2, func=Exp)

            Q = HD // 4  # 512 cols per sub-matmul
            for q in range(4):
                nc.tensor.matmul(out=psumA[q * 32:(q + 1) * 32, :],
                                 lhsT=tW, rhs=e1[:, q * Q:(q + 1) * Q],
                                 start=True, stop=True, tile_position=(0, q * 32))
            for q in range(4):
                nc.tensor.matmul(out=psumB[q * 32:(q + 1) * 32, :],
                                 lhsT=tW, rhs=e2[:, q * Q:(q + 1) * Q],
                                 start=True, stop=True, tile_position=(0, q * 32))

            # mask-dots: (Trep == cvec) * P, one stt per chunk
            scr = w.tile([P, HD], BF16, name="scr")
            nc.vector.scalar_tensor_tensor(
                out=scr, in0=tT1, scalar=tC[:, 0:1], in1=tP1,
                op0=mybir.AluOpType.is_equal, op1=mybir.AluOpType.mult,
                accum_out=res[:, 0:1],
            )
            nc.vector.scalar_tensor_tensor(
                out=scr, in0=tT2, scalar=tC[:, 0:1], in1=tP2,
                op0=mybir.AluOpType.is_equal, op1=mybir.AluOpType.mult,
                accum_out=res[:, 1:2],
            )

            lnout = w.tile([P, Q], BF16, name="lnout")
            nc.scalar.activation(out=lnout, in_=psumA, func=Ln,
                                 accum_out=res[:, 2:3])
            nc.scalar.activation(out=lnout, in_=psumB, func=Ln,
                                 accum_out=res[:, 3:4])

            nc.sync.dma_start(out=out, in_=res)
    if finalize:
        nc.finalize()
    return nc


_NC_CACHE = {}


def _get_nc():
    if "nc" not in _NC_CACHE:
        _NC_CACHE["nc"] = build_nc()
    return _NC_CACHE["nc"]


def prep_inputs(preds, targets):
    """Layout B: core k gets flat pixels [k*131072, (k+1)*131072) as
    [32 rows, 4096 cols]; partition c*32+r = channel c, row r."""
    p = np.asarray(preds)[:, -1]       # (4, 4, 512, 512) f32
    t = np.asarray(targets)[:, -1]     # (4, 512, 512) int
    # (C, N_CORES, 32, 4096)
    arr = np.transpose(p, (1, 0, 2, 3)).reshape(C, N_CORES, 32, FD)
    arr = arr.astype(ml_dtypes.float8_e4m3)
    tf = t.reshape(N_CORES, 32, FD).astype(ml_dtypes.float8_e4m3)
    W = np.zeros((P, 32), dtype=ml_dtypes.bfloat16)
    for pp in range(P):
        W[pp, pp % 32] = 1.0
    cvec = (np.arange(P, dtype=np.float32) // 32)[:, None]
    maps = []
    for k in range(N_CORES):
        pb = arr[:, k].reshape(P, FD)          # partition c*32+r
        trep = np.tile(tf[k], (4, 1))          # [128, 4096]
        maps.append({
            "inpP1": np.ascontiguousarray(pb[:, 0:HD]),
            "inpP2": np.ascontiguousarray(pb[:, HD:FD]),
            "inpT1": np.ascontiguousarray(trep[:, 0:HD]),
            "inpT2": np.ascontiguousarray(trep[:, HD:FD]),
            "inpW": W,
            "inpC": cvec,
        })
    return maps


def reduce_outputs(results):
    total = 0.0
    for d in results:
        o = d["out"].astype(np.float64)
        total += float(o[:, 2:4].sum() - o[:, 0:2].sum())
    return np.float32(total / N_BATCH)


def kernel(preds, targets, _trace=False, _trace_kwargs=None):
    from concourse.bass_utils import run_bass_kernel_spmd

    in_maps = prep_inputs(preds, targets)
    nc = _get_nc()
    r = run_bass_kernel_spmd(
        nc, in_maps, core_ids=list(range(N_CORES)),
        trace=_trace, **(_trace_kwargs or {}),
    )
    kernel.last_run = r
    return reduce_outputs(r.results)


kernel.last_run = None
